# revision 18
# baseline (speedup 1.0000x reference)
"""MultiHeadedAttention Trainium2 kernel.

Problem: B=2, T=2048, D=1024, H=16 heads (DK=64), fp32 in/out, padding mask
on keys. out = softmax(mask(QWq (KWk)^T / 8)) @ (VWv) @ Wo^T + biases.

Sharding (8 cores): core c -> batch b = c//4, head group g = c%4 (4 heads,
256 projection columns). Each core computes its heads' attention and a
partial output projection; host sums the 4 partials per batch (+ bo).

The run is ScalarE-bound: 64 key-chunk slots x 2 exp ACTIVATEs (~1.3us each
with sem overhead) = ~166us floor. v3 minimizes time outside that window:
  - consolidated activations (one SBUF tensor per input, few sub-DMAs for
    pacing) so the Sync queue isn't serialized on ~50 descriptor issues.
  - DMA order = dependency order of the first exp: wk, xk, wq, xq[:1024],
    mask, then xq[1024:], wv, xv, wo.
  - zero-bias fast path (the graded problem has bq=bk=bv=0): projection
    PSUM results are copied (ScalarE for kT m=0 which is exp-critical,
    VectorE otherwise) instead of bias-added, with no bias-DMA dependency.
  - kproj m=1 runs between kproj m=0 and qproj th=0 (its xk input is
    resident; it overlaps the xq front-half DMA stream).
  - riders inside unit 0's slots: qproj th=1 (slots 3-6), vproj (slots
    8-15); outproj qh=0 rides unit 3 (kc%2).
  - per unit, the previous unit's V matmuls run at slots 0-13 with a double
    group at 14, so the softmax-normalize chain (DVE+GpSimd) finishes
    before the next unit needs its PSUM slots back.
  - tail: AV half 0 -> norm -> AV half 1 with outproj trs 0-3 interleaved
    -> norm -> trs 4-7. Output DMA is bf16 (host accumulates in fp32).
"""

import numpy as np
import ml_dtypes

import concourse.bass as bass
import concourse.bacc as bacc
import concourse.tile as tile
from concourse import mybir
from concourse.bass_utils import run_bass_kernel_spmd

B, T, D, H = 2, 2048, 1024, 16
DK = D // H  # 64
GH = 4       # heads per core
GC = GH * DK  # 256 proj columns per core
NCORES = 8
KC = T // 128   # 16 key chunks
DCH = D // 128  # 8 contraction chunks
F32 = mybir.dt.float32
BF16 = mybir.dt.bfloat16

MASK_NEG = -30000.0


def build_program(with_bias: bool):
    nc = bacc.Bacc("TRN2")

    # ---- DRAM parameters (per-core shapes) ----
    xq_d = nc.declare_dram_parameter("xq", [128, DCH, T], BF16, isOutput=False)
    xk_d = nc.declare_dram_parameter("xk", [128, DCH, T], BF16, isOutput=False)
    xv_d = nc.declare_dram_parameter("xv", [128, DCH, T], BF16, isOutput=False)
    wq_d = nc.declare_dram_parameter("wq", [128, DCH, GC], BF16, isOutput=False)
    wk_d = nc.declare_dram_parameter("wk", [128, DCH, GC], BF16, isOutput=False)
    wv_d = nc.declare_dram_parameter("wv", [128, DCH, GC], BF16, isOutput=False)
    wo_d = nc.declare_dram_parameter("wo", [128, 2, D], BF16, isOutput=False)
    mask_d = nc.declare_dram_parameter("maskb", [128, KC], F32, isOutput=False)
    bq_d = nc.declare_dram_parameter("bq", [128, 2], F32, isOutput=False)
    bk_d = nc.declare_dram_parameter("bk", [128, 2], F32, isOutput=False)
    bv_d = nc.declare_dram_parameter("bv", [64, GH], F32, isOutput=False)
    out_d = nc.declare_dram_parameter("out", [T, D], BF16, isOutput=True)

    with tile.TileContext(nc) as tc:
        with (
            tc.tile_pool(name="persist", bufs=1) as pp,
            tc.tile_pool(name="psum", bufs=4, space="PSUM") as psp,
        ):
            # persistent sbuf tensors
            wq_sb = pp.tile([128, DCH, GC], BF16, tag="wq")
            wk_sb = pp.tile([128, DCH, GC], BF16, tag="wk")
            wv_sb = pp.tile([128, DCH, GC], BF16, tag="wv")
            wo_sb = pp.tile([128, 2, D], BF16, tag="wo")
            mask_sb = pp.tile([128, KC], F32, tag="mask")
            qT_sb = pp.tile([128, 2, T], BF16, tag="qT")
            kT_sb = pp.tile([128, 2, T], BF16, tag="kT")
            v_sb = pp.tile([128, KC, GH, 66], BF16, tag="v")
            xh_sb = [pp.tile([128, 2, 1024], BF16, tag=f"xh{q}", name=f"xh{q}")
                     for q in (0, 1)]
            if with_bias:
                bq_sb = pp.tile([128, 2], F32, tag="bq")
                bk_sb = pp.tile([128, 2], F32, tag="bk")
                bv_sb = pp.tile([64, GH], F32, tag="bv")
            nc.vector.memset(v_sb[:, :, :, 64:65], 1.0)

            xvp_cm = tc.tile_pool(name="xv", bufs=1)
            xvp = xvp_cm.__enter__()
            xv_sb = xvp.tile([128, DCH, T], BF16, tag="xv")
            xqp_cm = tc.tile_pool(name="xq", bufs=1)
            xqp = xqp_cm.__enter__()
            xq_sb = xqp.tile([128, DCH, T], BF16, tag="xq")

            def proj_store(dst, src, engine):
                if engine == "scalar":
                    nc.scalar.copy(dst, src)
                else:
                    nc.vector.tensor_copy(dst, src)

            # ---- prologue DMAs + k projection (DMA-paced) + q-proj th=0 ----
            with tc.tile_pool(name="xk", bufs=1) as xkp:
                xk_sb = xkp.tile([128, DCH, T], BF16, tag="xk")
                nc.sync.dma_start(out=wk_sb[:], in_=wk_d[:])
                for k2 in range(4):  # paced: 2 chunks per DMA
                    nc.sync.dma_start(out=xk_sb[:, 2 * k2:2 * k2 + 2, :],
                                      in_=xk_d[:, 2 * k2:2 * k2 + 2, :])
                nc.sync.dma_start(out=wq_sb[:], in_=wq_d[:])
                for k2 in range(4):
                    nc.sync.dma_start(
                        out=xq_sb[:, 2 * k2:2 * k2 + 2, 0:1024],
                        in_=xq_d[:, 2 * k2:2 * k2 + 2, 0:1024])
                nc.sync.dma_start(out=mask_sb[:], in_=mask_d[:])
                if with_bias:
                    nc.sync.dma_start(out=bq_sb[:], in_=bq_d[:])
                    nc.sync.dma_start(out=bk_sb[:], in_=bk_d[:])
                    nc.sync.dma_start(out=bv_sb[:], in_=bv_d[:])
                for k2 in range(2):
                    nc.sync.dma_start(
                        out=xq_sb[:, 4 * k2:4 * k2 + 4, 1024:2048],
                        in_=xq_d[:, 4 * k2:4 * k2 + 4, 1024:2048])
                nc.sync.dma_start(out=wv_sb[:], in_=wv_d[:])
                for k2 in range(2):
                    nc.sync.dma_start(out=xv_sb[:, 4 * k2:4 * k2 + 4, :],
                                      in_=xv_d[:, 4 * k2:4 * k2 + 4, :])
                nc.sync.dma_start(out=wo_sb[:], in_=wo_d[:])

                # k projection: m=0 (exp-critical), then m=1 (xk resident,
                # overlaps the xq front-half DMA stream). PSUM results are
                # stored via ScalarE (idle until the first exp) so the
                # pool slots recycle fast — se tiles of the first key-chunk
                # slots reuse them and must not wait on the Vector queue.
                for m in range(2):
                    pst = [psp.tile([128, 1024], F32, tag="ps", name="ps")
                           for _ in range(2)]
                    for k in range(DCH):
                        for th in range(2):
                            for n in range(2):
                                nc.tensor.matmul(
                                    pst[th][:, n * 512:(n + 1) * 512],
                                    wk_sb[:, k, m * 128:(m + 1) * 128],
                                    xk_sb[:, k, th * 1024 + n * 512:
                                          th * 1024 + (n + 1) * 512],
                                    start=(k == 0), stop=(k == DCH - 1),
                                    skip_group_check=True,
                                )
                    for th in range(2):
                        dst = kT_sb[:, m, th * 1024:(th + 1) * 1024]
                        if with_bias:
                            nc.vector.tensor_scalar_add(
                                dst, pst[th][:], bk_sb[:, m:m + 1])
                        else:
                            proj_store(dst, pst[th][:], "scalar")

                # q projection for token half th=0 (both m) -> unit (0,0)
                pst = [psp.tile([128, 1024], F32, tag="ps", name="ps")
                       for _ in range(2)]
                for k in range(DCH):
                    for m in range(2):
                        for n in range(2):
                            nc.tensor.matmul(
                                pst[m][:, n * 512:(n + 1) * 512],
                                wq_sb[:, k, m * 128:(m + 1) * 128],
                                xq_sb[:, k, n * 512:(n + 1) * 512],
                                start=(k == 0), stop=(k == DCH - 1),
                                skip_group_check=True,
                            )
                for m in range(2):
                    dst = qT_sb[:, m, 0:1024]
                    if with_bias:
                        nc.vector.tensor_scalar_add(
                            dst, pst[m][:], bq_sb[:, m:m + 1])
                    else:
                        proj_store(dst, pst[m][:], "vector")

            # ---- attention units, software-pipelined ----
            bc_pools = (
                tc.tile_pool(name="expp", bufs=32),
                tc.tile_pool(name="outp", bufs=3),
                tc.tile_pool(name="normp", bufs=2),
            )
            exp_pool = bc_pools[0].__enter__()
            out_pool = bc_pools[1].__enter__()
            norm_pool = bc_pools[2].__enter__()

            def emit_vproj(tcn):
                ps = psp.tile([128, GH, 64], F32, tag="ps", name="vps")
                for k in range(DCH):
                    nc.tensor.matmul(
                        ps[:],
                        xv_sb[:, k, tcn * 128:(tcn + 1) * 128],
                        wv_sb[:, k, :],
                        start=(k == 0), stop=(k == DCH - 1),
                        skip_group_check=True,
                    )
                nc.vector.tensor_copy(v_sb[:, tcn, :, 0:64], ps[:])

            def emit_v(prev, kc):
                qh, pr, o2, exs = prev
                for hh in range(2):
                    h = 2 * pr + hh
                    for n in range(2):
                        nc.tensor.matmul(
                            o2[hh][:, n * 512:(n + 1) * 512],
                            v_sb[:, kc, h, 0:65],
                            exs[kc][hh][:, n * 512:(n + 1) * 512],
                            start=(kc == 0), stop=(kc == KC - 1),
                            skip_group_check=True,
                        )

            def emit_norm(prev, half=None):
                qh, pr, o2, exs = prev
                sl = slice(0, 1024) if half is None else \
                    slice(half * 512, (half + 1) * 512)
                w = sl.stop - sl.start
                for hh in range(2):
                    rr = norm_pool.tile([1, 2, 1024], F32, tag="rr", name="rr")
                    nc.vector.tensor_copy(rr[:, 0, :w], o2[hh][64:65, sl])
                    nc.vector.reciprocal_approx_fast(rr[:, 1, :w], rr[:, 0, :w])
                    rb = norm_pool.tile([64, 1024], F32, tag="rb", name="rb",
                                        bufs=1)
                    nc.gpsimd.partition_broadcast(rb[:, :w], rr[:, 1, :w])
                    if hh == 0:
                        nc.vector.tensor_mul(
                            xh_sb[qh][0:64, pr, sl], o2[hh][0:64, sl],
                            rb[:, :w])
                        if with_bias:
                            nc.vector.tensor_scalar_add(
                                xh_sb[qh][0:64, pr, sl],
                                xh_sb[qh][0:64, pr, sl],
                                bv_sb[:, 2 * pr:2 * pr + 1])
                    else:
                        tmp = norm_pool.tile([64, 1024], BF16, tag="tmp",
                                             name="tmp", bufs=1)
                        nc.vector.tensor_mul(tmp[:, :w], o2[hh][0:64, sl],
                                             rb[:, :w])
                        if with_bias:
                            nc.vector.tensor_scalar_add(
                                tmp[:, :w], tmp[:, :w],
                                bv_sb[:, 2 * pr + 1:2 * pr + 2])
                        nc.sync.dma_start(
                            out=xh_sb[qh][64:128, pr, sl], in_=tmp[:, :w])

            ot_state = [None]

            def emit_outproj(qh, tr, tail=False):
                tcn = qh * 8 + tr
                po = psp.tile([128, 1024], F32, tag="ps", name="po")
                for m in range(2):
                    for n in range(2):
                        nc.tensor.matmul(
                            po[:, n * 512:(n + 1) * 512],
                            xh_sb[qh][:, m, tr * 128:(tr + 1) * 128],
                            wo_sb[:, m, n * 512:(n + 1) * 512],
                            start=(m == 0), stop=(m == 1),
                            skip_group_check=True,
                        )
                if ot_state[0] is None:
                    ot_state[0] = out_pool.tile([128, 2, 1024], BF16,
                                                tag="ot", name="ot", bufs=2)
                ot = ot_state[0]
                # in the tail ScalarE is idle: alternate copies across engines
                if tail and tr % 2 == 0:
                    nc.scalar.copy(ot[:, tr % 2, :], po[:])
                else:
                    nc.vector.tensor_copy(ot[:, tr % 2, :], po[:])
                if tr % 2 == 1:
                    base = (tcn - 1) * 128
                    nc.sync.dma_start(
                        out=out_d[base:base + 256, :].rearrange(
                            "(t p) c -> p t c", t=2),
                        in_=ot[:])
                    ot_state[0] = None

            kqpst = [None]

            def emit_qproj_th1(part):
                # m-serialized on ONE psum tile so se keeps 3 of 4 slots:
                # parts 0,1: m=0 k-halves; 2: m=0 store; 3,4: m=1; 5: store
                m, sub = part // 3, part % 3
                if part == 0:
                    kqpst[0] = psp.tile([128, 1024], F32, tag="ps", name="qp")
                if sub < 2:
                    for k in range(4 * sub, 4 * sub + 4):
                        for n in range(2):
                            nc.tensor.matmul(
                                kqpst[0][:, n * 512:(n + 1) * 512],
                                wq_sb[:, k, m * 128:(m + 1) * 128],
                                xq_sb[:, k, 1024 + n * 512:
                                      1024 + (n + 1) * 512],
                                start=(k == 0), stop=(k == 7),
                                skip_group_check=True,
                            )
                else:
                    dst = qT_sb[:, m, 1024:2048]
                    if with_bias:
                        nc.vector.tensor_scalar_add(
                            dst, kqpst[0][:], bq_sb[:, m:m + 1])
                    else:
                        proj_store(dst, kqpst[0][:], "vector")
                    if m == 1:
                        kqpst[0] = None

            # Half-unit V lag: unit u's V matmuls for kc 0-7 run in its own
            # slots 8-15; kc 8-15 run in unit u+1's slots 0-7 (u0's 16
            # groups all ride u1's slots 0-7 at 2/slot since u0's slots
            # hold the projection/vproj riders). norm(u) sits at slot 8 of
            # u+1, right when the o2 PSUM handoff happens. The tail is only
            # u3's kc 8-15 V accumulation.
            units = [(0, 0), (0, 1), (1, 0), (1, 1)]
            ustate = []
            for ui, (qh, pr) in enumerate(units):
                q0 = qh * 1024
                o2 = [psp.tile([65, 1024], F32, tag="ps", name="o2")
                      for _ in range(2)]
                exs = []
                cur = (qh, pr, o2, exs)
                ustate.append(cur)
                for kc in range(KC):
                    se = [psp.tile([128, 1024], F32, tag="ps", name="se")
                          for _ in range(2)]
                    # scores first: the exp ACTIVATE depends on these
                    for hh in range(2):
                        pb = 64 * hh
                        for n in range(2):
                            nc.tensor.matmul(
                                se[hh][:, n * 512:(n + 1) * 512],
                                kT_sb[pb:pb + 64, pr,
                                      kc * 128:(kc + 1) * 128],
                                qT_sb[pb:pb + 64, pr,
                                      q0 + n * 512:q0 + (n + 1) * 512],
                                start=True, stop=True,
                            )
                    # V matmul schedule (see comment above)
                    if ui == 1 and kc < 8:
                        emit_v(ustate[0], 2 * kc)
                        emit_v(ustate[0], 2 * kc + 1)
                    elif ui >= 2 and kc < 8:
                        emit_v(ustate[ui - 1], kc + 8)
                    elif ui >= 1 and kc >= 8:
                        if kc == 8:
                            emit_norm(ustate[ui - 1])
                        emit_v(cur, kc - 8)
                    # riders
                    if ui == 0:
                        if 2 <= kc <= 4:
                            emit_qproj_th1(kc - 2)
                        elif 6 <= kc <= 8:
                            emit_qproj_th1(kc - 3)
                        if kc >= 8:
                            emit_vproj(2 * (kc - 8))
                            emit_vproj(2 * (kc - 8) + 1)
                    elif ui == 2 and kc >= 9:
                        emit_outproj(0, kc - 9)
                    elif ui == 3 and kc == 0:
                        emit_outproj(0, 7)
                    ex = [exp_pool.tile([128, 1024], BF16, tag="ex", name="ex")
                          for _ in range(2)]
                    for hh in range(2):
                        nc.scalar.activation(
                            ex[hh][:], se[hh][:],
                            mybir.ActivationFunctionType.Exp,
                            bias=mask_sb[:, kc:kc + 1],
                            scale=float(DK) ** -0.5,
                        )
                    exs.append(ex)

            # tail: u3's kc 8-15 V accumulation, split by q-half so outproj
            # trs 0-3 interleave with the half-1 matmuls.
            last = ustate[3]
            for half in range(2):
                n0 = half * 512
                for kc in range(8, KC):
                    for hh in range(2):
                        nc.tensor.matmul(
                            last[2][hh][:, n0:n0 + 512],
                            v_sb[:, kc, 2 * last[1] + hh, 0:65],
                            last[3][kc][hh][:, n0:n0 + 512],
                            start=False, stop=(kc == KC - 1),
                            skip_group_check=True,
                        )
                    if half == 1 and kc % 2 == 1:
                        emit_outproj(1, (kc - 9) // 2, tail=True)
                emit_norm(last, half=half)
            for tr in range(4, 8):
                emit_outproj(1, tr, tail=True)

            for _p in reversed(bc_pools):
                _p.__exit__(None, None, None)
            xqp_cm.__exit__(None, None, None)
            xvp_cm.__exit__(None, None, None)

    nc.compile()
    return nc


_CACHE = {}


def _get_program(with_bias: bool):
    if with_bias not in _CACHE:
        _CACHE[with_bias] = build_program(with_bias)
    return _CACHE[with_bias]


def make_in_maps(query, key, value, mask, Wq, bq, Wk, bk, Wv, bv, Wo, bo):
    bf = ml_dtypes.bfloat16
    # transposed bf16 activations, [128, DCH, T], shared per batch
    xt = {}
    for nm, x in (("xq", query), ("xk", key), ("xv", value)):
        for b in range(B):
            xt[nm, b] = np.ascontiguousarray(
                x[b].T.reshape(DCH, 128, T).transpose(1, 0, 2)).astype(bf)
    in_maps = []
    for c in range(NCORES):
        b, g = c // 4, c % 4
        cols = slice(GC * g, GC * (g + 1))
        m = {}
        for nm in ("xq", "xk", "xv"):
            m[nm] = xt[nm, b]
        for nm, W in (("wq", Wq), ("wk", Wk), ("wv", Wv)):
            m[nm] = np.ascontiguousarray(
                W[cols, :].T.reshape(DCH, 128, GC).transpose(1, 0, 2)
            ).astype(bf)
        m["wo"] = np.ascontiguousarray(
            Wo[:, cols].T.reshape(2, 128, D).transpose(1, 0, 2)).astype(bf)
        mb = np.where(mask[b, 0] != 0, 0.0, MASK_NEG).astype(np.float32)
        m["maskb"] = np.ascontiguousarray(mb.reshape(KC, 128).T)
        m["bq"] = np.ascontiguousarray(
            bq[cols].reshape(2, 128).T.astype(np.float32))
        m["bk"] = np.ascontiguousarray(
            bk[cols].reshape(2, 128).T.astype(np.float32))
        m["bv"] = np.ascontiguousarray(
            bv[cols].reshape(GH, 64).T.astype(np.float32))
        in_maps.append(m)
    return in_maps


def kernel(query, key, value, mask, Wq, bq, Wk, bk, Wv, bv, Wo, bo,
           _trace=False):
    query, key, value = (np.asarray(a, np.float32) for a in (query, key, value))
    mask = np.asarray(mask)
    with_bias = bool(np.any(np.asarray(bq)) or np.any(np.asarray(bk))
                     or np.any(np.asarray(bv)))
    nc = _get_program(with_bias)
    in_maps = make_in_maps(query, key, value, mask, Wq, bq, Wk, bk, Wv, bv,
                           Wo, bo)
    res = run_bass_kernel_spmd(nc, in_maps, list(range(NCORES)), trace=_trace)
    out = np.zeros((B, T, D), np.float32)
    for c in range(NCORES):
        out[c // 4] += res.results[c]["out"].astype(np.float32)
    out += np.asarray(bo, np.float32)[None, None, :]
    if _trace:
        kernel.last_exec_time_ns = res.exec_time_ns
        kernel.last_results = res
    return out


# revision 22
# speedup vs baseline: 1.1742x; 1.1742x over previous
"""MultiHeadedAttention Trainium2 kernel.

Problem: B=2, T=2048, D=1024, H=16 heads (DK=64), fp32 in/out, padding mask
on keys. out = softmax(mask(QWq (KWk)^T / 8)) @ (VWv) @ Wo^T + biases.

Sharding (8 cores): core c -> batch b = c//4, head group g = c%4 (4 heads,
256 projection columns). Each core computes its heads' attention and a
partial output projection; host sums the 4 partials per batch (+ bo).

The run is ScalarE-bound: 64 key-chunk slots x 2 exp ACTIVATEs (~1.3us each
with sem overhead) = ~166us floor. v3 minimizes time outside that window:
  - consolidated activations (one SBUF tensor per input, few sub-DMAs for
    pacing) so the Sync queue isn't serialized on ~50 descriptor issues.
  - DMA order = dependency order of the first exp: wk, xk, wq, xq[:1024],
    mask, then xq[1024:], wv, xv, wo.
  - zero-bias fast path (the graded problem has bq=bk=bv=0): projection
    PSUM results are copied (ScalarE for kT m=0 which is exp-critical,
    VectorE otherwise) instead of bias-added, with no bias-DMA dependency.
  - kproj m=1 runs between kproj m=0 and qproj th=0 (its xk input is
    resident; it overlaps the xq front-half DMA stream).
  - riders inside unit 0's slots: qproj th=1 (slots 3-6), vproj (slots
    8-15); outproj qh=0 rides unit 3 (kc%2).
  - per unit, the previous unit's V matmuls run at slots 0-13 with a double
    group at 14, so the softmax-normalize chain (DVE+GpSimd) finishes
    before the next unit needs its PSUM slots back.
  - tail: AV half 0 -> norm -> AV half 1 with outproj trs 0-3 interleaved
    -> norm -> trs 4-7. Output DMA is bf16 (host accumulates in fp32).
"""

import numpy as np
import ml_dtypes

import concourse.bass as bass
import concourse.bacc as bacc
import concourse.tile as tile
from concourse import mybir
from concourse.bass_utils import run_bass_kernel_spmd

B, T, D, H = 2, 2048, 1024, 16
DK = D // H  # 64
GH = 4       # heads per core
GC = GH * DK  # 256 proj columns per core
NCORES = 8
KC = T // 128   # 16 key chunks
DCH = D // 128  # 8 contraction chunks
F32 = mybir.dt.float32
BF16 = mybir.dt.bfloat16

MASK_NEG = -30000.0


def build_program(with_bias: bool):
    nc = bacc.Bacc("TRN2")

    # ---- DRAM parameters (per-core shapes) ----
    xq_d = nc.declare_dram_parameter("xq", [128, DCH, T], BF16, isOutput=False)
    xk_d = nc.declare_dram_parameter("xk", [128, DCH, T], BF16, isOutput=False)
    xv_d = nc.declare_dram_parameter("xv", [128, DCH, T], BF16, isOutput=False)
    wq_d = nc.declare_dram_parameter("wq", [128, DCH, GC], BF16, isOutput=False)
    wk_d = nc.declare_dram_parameter("wk", [128, DCH, GC], BF16, isOutput=False)
    wv_d = nc.declare_dram_parameter("wv", [128, DCH, GC], BF16, isOutput=False)
    wo_d = nc.declare_dram_parameter("wo", [128, 2, D], BF16, isOutput=False)
    mask_d = nc.declare_dram_parameter("maskb", [128, KC], F32, isOutput=False)
    bq_d = nc.declare_dram_parameter("bq", [128, 2], F32, isOutput=False)
    bk_d = nc.declare_dram_parameter("bk", [128, 2], F32, isOutput=False)
    bv_d = nc.declare_dram_parameter("bv", [64, GH], F32, isOutput=False)
    out_d = nc.declare_dram_parameter("out", [T, D], BF16, isOutput=True)

    with tile.TileContext(nc) as tc:
        with (
            tc.tile_pool(name="persist", bufs=1) as pp,
            tc.tile_pool(name="psum", bufs=4, space="PSUM") as psp,
        ):
            # persistent sbuf tensors
            wq_sb = pp.tile([128, DCH, GC], BF16, tag="wq")
            wk_sb = pp.tile([128, DCH, GC], BF16, tag="wk")
            wv_sb = pp.tile([128, DCH, GC], BF16, tag="wv")
            wo_sb = pp.tile([128, 2, D], BF16, tag="wo")
            mask_sb = pp.tile([128, KC], F32, tag="mask")
            qT_sb = pp.tile([128, 2, T], BF16, tag="qT")
            kT_sb = pp.tile([128, 2, T], BF16, tag="kT")
            v_sb = pp.tile([128, KC, GH, 66], BF16, tag="v")
            xh_sb = [pp.tile([128, 2, 1024], BF16, tag=f"xh{q}", name=f"xh{q}")
                     for q in (0, 1)]
            if with_bias:
                bq_sb = pp.tile([128, 2], F32, tag="bq")
                bk_sb = pp.tile([128, 2], F32, tag="bk")
                bv_sb = pp.tile([64, GH], F32, tag="bv")
            nc.vector.memset(v_sb[:, :, :, 64:65], 1.0)

            xvp_cm = tc.tile_pool(name="xv", bufs=1)
            xvp = xvp_cm.__enter__()
            xv_sb = xvp.tile([128, DCH, T], BF16, tag="xv")
            xqp_cm = tc.tile_pool(name="xq", bufs=1)
            xqp = xqp_cm.__enter__()
            xq_sb = xqp.tile([128, DCH, T], BF16, tag="xq")

            def proj_store(dst, src, engine):
                if engine == "scalar":
                    nc.scalar.copy(dst, src)
                else:
                    nc.vector.tensor_copy(dst, src)

            # ---- prologue DMAs + k projection (DMA-paced) + q-proj th=0 ----
            with tc.tile_pool(name="xk", bufs=1) as xkp:
                xk_sb = xkp.tile([128, DCH, T], BF16, tag="xk")
                nc.sync.dma_start(out=wk_sb[:], in_=wk_d[:])
                for k2 in range(4):  # paced: 2 chunks per DMA
                    nc.sync.dma_start(out=xk_sb[:, 2 * k2:2 * k2 + 2, :],
                                      in_=xk_d[:, 2 * k2:2 * k2 + 2, :])
                nc.sync.dma_start(out=wq_sb[:], in_=wq_d[:])
                for k2 in range(4):
                    nc.sync.dma_start(
                        out=xq_sb[:, 2 * k2:2 * k2 + 2, 0:1024],
                        in_=xq_d[:, 2 * k2:2 * k2 + 2, 0:1024])
                nc.sync.dma_start(out=mask_sb[:], in_=mask_d[:])
                if with_bias:
                    nc.sync.dma_start(out=bq_sb[:], in_=bq_d[:])
                    nc.sync.dma_start(out=bk_sb[:], in_=bk_d[:])
                    nc.sync.dma_start(out=bv_sb[:], in_=bv_d[:])
                for k2 in range(2):
                    nc.sync.dma_start(
                        out=xq_sb[:, 4 * k2:4 * k2 + 4, 1024:2048],
                        in_=xq_d[:, 4 * k2:4 * k2 + 4, 1024:2048])
                nc.sync.dma_start(out=wv_sb[:], in_=wv_d[:])
                for k2 in range(2):
                    nc.sync.dma_start(out=xv_sb[:, 4 * k2:4 * k2 + 4, :],
                                      in_=xv_d[:, 4 * k2:4 * k2 + 4, :])
                nc.sync.dma_start(out=wo_sb[:], in_=wo_d[:])

                # k projection: m=0 (exp-critical), then m=1 (xk resident,
                # overlaps the xq front-half DMA stream). PSUM results are
                # stored via ScalarE (idle until the first exp) so the
                # pool slots recycle fast — se tiles of the first key-chunk
                # slots reuse them and must not wait on the Vector queue.
                for m in range(2):
                    pst = [psp.tile([128, 1024], F32, tag="ps", name="ps")
                           for _ in range(2)]
                    for k in range(DCH):
                        for th in range(2):
                            for n in range(2):
                                nc.tensor.matmul(
                                    pst[th][:, n * 512:(n + 1) * 512],
                                    wk_sb[:, k, m * 128:(m + 1) * 128],
                                    xk_sb[:, k, th * 1024 + n * 512:
                                          th * 1024 + (n + 1) * 512],
                                    start=(k == 0), stop=(k == DCH - 1),
                                    skip_group_check=True,
                                )
                    for th in range(2):
                        dst = kT_sb[:, m, th * 1024:(th + 1) * 1024]
                        if with_bias:
                            nc.vector.tensor_scalar_add(
                                dst, pst[th][:], bk_sb[:, m:m + 1])
                        else:
                            proj_store(dst, pst[th][:], "scalar")

                # q projection for token half th=0 (both m) -> unit (0,0)
                pst = [psp.tile([128, 1024], F32, tag="ps", name="ps")
                       for _ in range(2)]
                for k in range(DCH):
                    for m in range(2):
                        for n in range(2):
                            nc.tensor.matmul(
                                pst[m][:, n * 512:(n + 1) * 512],
                                wq_sb[:, k, m * 128:(m + 1) * 128],
                                xq_sb[:, k, n * 512:(n + 1) * 512],
                                start=(k == 0), stop=(k == DCH - 1),
                                skip_group_check=True,
                            )
                for m in range(2):
                    dst = qT_sb[:, m, 0:1024]
                    if with_bias:
                        nc.vector.tensor_scalar_add(
                            dst, pst[m][:], bq_sb[:, m:m + 1])
                    else:
                        proj_store(dst, pst[m][:], "vector")

            # ---- attention units, software-pipelined ----
            bc_pools = (
                tc.tile_pool(name="expp", bufs=32),
                tc.tile_pool(name="outp", bufs=3),
                tc.tile_pool(name="normp", bufs=2),
            )
            exp_pool = bc_pools[0].__enter__()
            out_pool = bc_pools[1].__enter__()
            norm_pool = bc_pools[2].__enter__()

            def emit_vproj(tcn):
                ps = psp.tile([128, GH, 64], F32, tag="ps", name="vps")
                for k in range(DCH):
                    nc.tensor.matmul(
                        ps[:],
                        xv_sb[:, k, tcn * 128:(tcn + 1) * 128],
                        wv_sb[:, k, :],
                        start=(k == 0), stop=(k == DCH - 1),
                        skip_group_check=True,
                    )
                nc.vector.tensor_copy(v_sb[:, tcn, :, 0:64], ps[:])

            def emit_v(prev, kc):
                qh, pr, o2, exs = prev
                for hh in range(2):
                    h = 2 * pr + hh
                    for n in range(2):
                        nc.tensor.matmul(
                            o2[hh][:, n * 512:(n + 1) * 512],
                            v_sb[:, kc, h, 0:65],
                            exs[kc][hh][:, n * 512:(n + 1) * 512],
                            start=(kc == 0), stop=(kc == KC - 1),
                            skip_group_check=True,
                        )

            def emit_norm(prev, half=None):
                qh, pr, o2, exs = prev
                sl = slice(0, 1024) if half is None else \
                    slice(half * 512, (half + 1) * 512)
                w = sl.stop - sl.start
                for hh in range(2):
                    rr = norm_pool.tile([1, 2, 1024], F32, tag="rr", name="rr")
                    nc.vector.tensor_copy(rr[:, 0, :w], o2[hh][64:65, sl])
                    nc.vector.reciprocal_approx_fast(rr[:, 1, :w], rr[:, 0, :w])
                    rb = norm_pool.tile([64, 1024], F32, tag="rb", name="rb",
                                        bufs=1)
                    nc.gpsimd.partition_broadcast(rb[:, :w], rr[:, 1, :w])
                    if hh == 0:
                        nc.vector.tensor_mul(
                            xh_sb[qh][0:64, pr, sl], o2[hh][0:64, sl],
                            rb[:, :w])
                        if with_bias:
                            nc.vector.tensor_scalar_add(
                                xh_sb[qh][0:64, pr, sl],
                                xh_sb[qh][0:64, pr, sl],
                                bv_sb[:, 2 * pr:2 * pr + 1])
                    else:
                        tmp = norm_pool.tile([64, 1024], BF16, tag="tmp",
                                             name="tmp")
                        nc.vector.tensor_mul(tmp[:, :w], o2[hh][0:64, sl],
                                             rb[:, :w])
                        if with_bias:
                            nc.vector.tensor_scalar_add(
                                tmp[:, :w], tmp[:, :w],
                                bv_sb[:, 2 * pr + 1:2 * pr + 2])
                        nc.sync.dma_start(
                            out=xh_sb[qh][64:128, pr, sl], in_=tmp[:, :w])

            def emit_outproj(qh, tr, tail=False):
                tcn = qh * 8 + tr
                po = psp.tile([128, 1024], F32, tag="ps", name="po")
                for m in range(2):
                    for n in range(2):
                        nc.tensor.matmul(
                            po[:, n * 512:(n + 1) * 512],
                            xh_sb[qh][:, m, tr * 128:(tr + 1) * 128],
                            wo_sb[:, m, n * 512:(n + 1) * 512],
                            start=(m == 0), stop=(m == 1),
                            skip_group_check=True,
                        )
                ot = out_pool.tile([128, 1024], BF16, tag="ot")
                # in the tail ScalarE is idle: alternate copies across engines
                if tail and tr % 2 == 0:
                    nc.scalar.copy(ot[:], po[:])
                else:
                    nc.vector.tensor_copy(ot[:], po[:])
                nc.sync.dma_start(
                    out=out_d[tcn * 128:(tcn + 1) * 128, :], in_=ot[:])

            kqpst = [None]

            def emit_qproj_th1(part):
                if part == 0:
                    kqpst[0] = [psp.tile([128, 1024], F32, tag="ps",
                                         name="qp") for _ in range(2)]
                if part < 4:
                    for k in (2 * part, 2 * part + 1):
                        for m in range(2):
                            for n in range(2):
                                nc.tensor.matmul(
                                    kqpst[0][m][:, n * 512:(n + 1) * 512],
                                    wq_sb[:, k, m * 128:(m + 1) * 128],
                                    xq_sb[:, k, 1024 + n * 512:
                                          1024 + (n + 1) * 512],
                                    start=(k == 0), stop=(k == 7),
                                    skip_group_check=True,
                                )
                else:
                    for m in range(2):
                        dst = qT_sb[:, m, 1024:2048]
                        if with_bias:
                            nc.vector.tensor_scalar_add(
                                dst, kqpst[0][m][:], bq_sb[:, m:m + 1])
                        else:
                            proj_store(dst, kqpst[0][m][:], "vector")
                    kqpst[0] = None

            # Half-unit V lag: unit u's V matmuls for kc 0-7 run in its own
            # slots 8-15; kc 8-15 run in unit u+1's slots 0-7 (u0's 16
            # groups all ride u1's slots 0-7 at 2/slot since u0's slots
            # hold the projection/vproj riders). norm(u) sits at slot 8 of
            # u+1, right when the o2 PSUM handoff happens. The tail is only
            # u3's kc 8-15 V accumulation.
            units = [(0, 0), (0, 1), (1, 0), (1, 1)]
            ustate = []
            for ui, (qh, pr) in enumerate(units):
                q0 = qh * 1024
                o2 = [psp.tile([65, 1024], F32, tag="ps", name="o2")
                      for _ in range(2)]
                exs = []
                cur = (qh, pr, o2, exs)
                ustate.append(cur)
                for kc in range(KC):
                    se = [psp.tile([128, 1024], F32, tag="ps", name="se")
                          for _ in range(2)]
                    # scores first: the exp ACTIVATE depends on these
                    for hh in range(2):
                        pb = 64 * hh
                        for n in range(2):
                            nc.tensor.matmul(
                                se[hh][:, n * 512:(n + 1) * 512],
                                kT_sb[pb:pb + 64, pr,
                                      kc * 128:(kc + 1) * 128],
                                qT_sb[pb:pb + 64, pr,
                                      q0 + n * 512:q0 + (n + 1) * 512],
                                start=True, stop=True,
                            )
                    # V matmul schedule (see comment above)
                    if ui == 1 and kc < 8:
                        emit_v(ustate[0], 2 * kc)
                        emit_v(ustate[0], 2 * kc + 1)
                    elif ui >= 2 and kc < 8:
                        emit_v(ustate[ui - 1], kc + 8)
                    elif ui >= 1 and kc >= 8:
                        if kc == 8:
                            emit_norm(ustate[ui - 1])
                        emit_v(cur, kc - 8)
                    # riders
                    if ui == 0:
                        if 2 <= kc <= 5:
                            emit_qproj_th1(kc - 2)
                        elif kc == 6:
                            emit_qproj_th1(4)
                        elif kc >= 8:
                            emit_vproj(2 * (kc - 8))
                            emit_vproj(2 * (kc - 8) + 1)
                    elif ui == 2 and kc >= 9:
                        emit_outproj(0, kc - 9)
                    elif ui == 3 and kc == 0:
                        emit_outproj(0, 7)
                    ex = [exp_pool.tile([128, 1024], BF16, tag="ex", name="ex")
                          for _ in range(2)]
                    for hh in range(2):
                        nc.scalar.activation(
                            ex[hh][:], se[hh][:],
                            mybir.ActivationFunctionType.Exp,
                            bias=mask_sb[:, kc:kc + 1],
                            scale=float(DK) ** -0.5,
                        )
                    exs.append(ex)

            # tail: u3's kc 8-15 V accumulation, split by q-half so outproj
            # trs 0-3 interleave with the half-1 matmuls.
            last = ustate[3]
            for half in range(2):
                n0 = half * 512
                for kc in range(8, KC):
                    for hh in range(2):
                        nc.tensor.matmul(
                            last[2][hh][:, n0:n0 + 512],
                            v_sb[:, kc, 2 * last[1] + hh, 0:65],
                            last[3][kc][hh][:, n0:n0 + 512],
                            start=False, stop=(kc == KC - 1),
                            skip_group_check=True,
                        )
                    if half == 1 and kc % 2 == 1:
                        emit_outproj(1, (kc - 9) // 2, tail=True)
                emit_norm(last, half=half)
            for tr in range(4, 8):
                emit_outproj(1, tr, tail=True)

            for _p in reversed(bc_pools):
                _p.__exit__(None, None, None)
            xqp_cm.__exit__(None, None, None)
            xvp_cm.__exit__(None, None, None)

    nc.compile()
    return nc


_CACHE = {}


def _get_program(with_bias: bool):
    if with_bias not in _CACHE:
        _CACHE[with_bias] = build_program(with_bias)
    return _CACHE[with_bias]


def make_in_maps(query, key, value, mask, Wq, bq, Wk, bk, Wv, bv, Wo, bo):
    bf = ml_dtypes.bfloat16
    # transposed bf16 activations, [128, DCH, T], shared per batch
    xt = {}
    for nm, x in (("xq", query), ("xk", key), ("xv", value)):
        for b in range(B):
            xt[nm, b] = np.ascontiguousarray(
                x[b].T.reshape(DCH, 128, T).transpose(1, 0, 2)).astype(bf)
    in_maps = []
    for c in range(NCORES):
        b, g = c // 4, c % 4
        cols = slice(GC * g, GC * (g + 1))
        m = {}
        for nm in ("xq", "xk", "xv"):
            m[nm] = xt[nm, b]
        for nm, W in (("wq", Wq), ("wk", Wk), ("wv", Wv)):
            m[nm] = np.ascontiguousarray(
                W[cols, :].T.reshape(DCH, 128, GC).transpose(1, 0, 2)
            ).astype(bf)
        m["wo"] = np.ascontiguousarray(
            Wo[:, cols].T.reshape(2, 128, D).transpose(1, 0, 2)).astype(bf)
        mb = np.where(mask[b, 0] != 0, 0.0, MASK_NEG).astype(np.float32)
        m["maskb"] = np.ascontiguousarray(mb.reshape(KC, 128).T)
        m["bq"] = np.ascontiguousarray(
            bq[cols].reshape(2, 128).T.astype(np.float32))
        m["bk"] = np.ascontiguousarray(
            bk[cols].reshape(2, 128).T.astype(np.float32))
        m["bv"] = np.ascontiguousarray(
            bv[cols].reshape(GH, 64).T.astype(np.float32))
        in_maps.append(m)
    return in_maps


def kernel(query, key, value, mask, Wq, bq, Wk, bk, Wv, bv, Wo, bo,
           _trace=False):
    query, key, value = (np.asarray(a, np.float32) for a in (query, key, value))
    mask = np.asarray(mask)
    with_bias = bool(np.any(np.asarray(bq)) or np.any(np.asarray(bk))
                     or np.any(np.asarray(bv)))
    nc = _get_program(with_bias)
    in_maps = make_in_maps(query, key, value, mask, Wq, bq, Wk, bk, Wv, bv,
                           Wo, bo)
    res = run_bass_kernel_spmd(nc, in_maps, list(range(NCORES)), trace=_trace)
    out = np.zeros((B, T, D), np.float32)
    for c in range(NCORES):
        out[c // 4] += res.results[c]["out"].astype(np.float32)
    out += np.asarray(bo, np.float32)[None, None, :]
    if _trace:
        kernel.last_exec_time_ns = res.exec_time_ns
        kernel.last_results = res
    return out
